# revision 50
# baseline (speedup 1.0000x reference)
"""GCN+MLP (ChebConv K=2, sym norm) Trainium2 Bass/Tile kernel.

nn_GCNMLP_81320910782821: out = MLP(relu(cheb1(relu(cheb0(embed(conv(x)))))))
with cheb(h) = h@W0 + (S@h)@W1 + b, S = -D^-1/2 A D^-1/2 (160k random edges,
E=10000 nodes, C=128 channels).

Sharding: data-parallel over batch B=32 -> 8 NeuronCores x 4 batch elems.
Per-core device kernel (bf16 compute, fp32 PSUM accumulation):
  * h kept SBUF-resident transposed: hT [C=128 part, 4b, E] bf16.
  * The segment-sum is one-hot matmul scatter over 128-edge chunks grouped
    by 128-row dest tiles; the one-hot matrices S_chunk[e,d] = w_e*(row_e==d)
    are host-precomputed and streamed from DRAM.
  * Layer 1 aggregates in x-space ((S@h0)@W1 == (S@X2)@(M1@W1)), so its
    messages are a pure input-layout transform that the host pre-gathers
    into edge order (em table) -> no device gather at all in layer 1.
  * Layer 2 messages are device-gathered from a row-major copy of h1 (h4,
    [E, 4b*C] bf16, 1024B rows) via dma_gather across 4 SWDGE queues.
  * Each dest tile's edges are split into two rounds by source node
    (< / >= MID): round-A of layer 2 only reads h4[:MID], so its gathers and
    scatters overlap the back half of layer 1; round partial sums round-trip
    through a DRAM buffer and are re-added on DVE.
  * Output uses the e = 79*p + j partition mapping so the final
    [N_PRED, E, PD] DMA has 1264B contiguous runs.

kernel(**inputs) takes FULL unsharded fp32/int64 inputs and returns the FULL
[B, N_PRED, E, PD] fp32 output. The Bass program is input-shape static but
depends on the per-tile chunk counts of the actual graph; it is built and
compiled on first call (cached per chunk signature).
"""

import functools

import numpy as np
import ml_dtypes

import concourse.bacc as bacc
import concourse.bass as bass
import concourse.mybir as mybir
import concourse.tile as tile
from concourse.bass_utils import run_bass_kernel_spmd

B, T, E, D = 32, 12, 10000, 4
C, H = 128, 64
N_PRED, PD = 12, 4
NCORES = 8
BPC = B // NCORES          # batch elems per core
P = 128
NJ = 79                    # e = NJ*p + j partition mapping
EP = P * NJ                # 10112 (E padded)
NT = EP // P               # 79 dest tiles of 128 rows
KD = T * D                 # 48 contraction dim of fused conv+embed
KDX = KD + 1               # + ones column carrying the fused bias
KP = 64                    # KDX padded (layer-1 message row 4*64*2B = 512B)
GG = 8                     # chunks per message group
OPD = N_PRED * PD          # 48
E_MAIN = (E // NJ) * NJ    # 9954 = 126*79 (rows covered by partitions 0..125)
MID = 40 * P               # 5120: source split point (h4-tile aligned)

BF = mybir.dt.bfloat16
F32 = mybir.dt.float32
I16 = mybir.dt.int16
AF = mybir.ActivationFunctionType
ALU = mybir.AluOpType
bf16 = ml_dtypes.bfloat16


# ---------------------------------------------------------------- host side

def _preprocess_graph(edge_index):
    """Sort edges by (dest tile, source round, dest row); pack into 128-edge
    chunks. Round A = sources < MID, round B = rest; within each round the
    chunk stream is dest-tile contiguous.

    Returns (idx_all [128, nch*8] i16, s_all [128, nch, 128] bf16,
             cols_rs [nch, 128] i64, chunk_tile tuple, n_round_a int).
    """
    row = np.asarray(edge_index[0], dtype=np.int64)
    col = np.asarray(edge_index[1], dtype=np.int64)
    deg = np.bincount(row, minlength=E).astype(np.float32)
    dis = np.where(deg > 0, 1.0 / np.sqrt(np.maximum(deg, 1.0)), 0.0).astype(np.float32)
    w = (-dis[row] * dis[col]).astype(np.float32)
    order = np.argsort(row, kind="stable")
    r_s, c_s, w_s = row[order], col[order], w[order]
    bounds = np.searchsorted(r_s, np.arange(NT + 1) * P)

    streams = {0: ([], [], [], []), 1: ([], [], [], [])}  # cols, rloc, ws, tile
    for t in range(NT):
        s, e_ = int(bounds[t]), int(bounds[t + 1])
        tc, tr, tw = c_s[s:e_], r_s[s:e_], w_s[s:e_]
        early = tc < MID
        for rnd, mask in ((0, early), (1, ~early)):
            cc, rr, ww = tc[mask], tr[mask], tw[mask]
            n = len(cc)
            nch_t = max(1, -(-n // P))
            pad = nch_t * P - n
            cols_p, rloc_p, ws_p, tl = streams[rnd]
            cols_p.append(np.pad(cc, (0, pad)))
            rloc_p.append(np.pad(rr - t * P, (0, pad)))
            ws_p.append(np.pad(ww, (0, pad)))
            tl += [t] * nch_t

    cols = np.concatenate(streams[0][0] + streams[1][0]).astype(np.int16)
    rloc = np.concatenate(streams[0][1] + streams[1][1]).astype(np.int64)
    ws = np.concatenate(streams[0][2] + streams[1][2]).astype(np.float32)
    chunk_tile = tuple(streams[0][3]) + tuple(streams[1][3])
    n_round_a = len(streams[0][3])
    nch = len(chunk_tile)
    # dma_gather index layout: idx i at [partition i%16, col i//16], x8 replicas
    idx_all = np.tile(cols.reshape(nch * 8, 16).T, (8, 1)).astype(np.int16)
    # one-hot scatter matrices, laid out [p(edge-in-chunk), chunk, dest]
    s_all = np.zeros((P, nch, P), dtype=bf16)
    cc, pp = np.meshgrid(np.arange(nch), np.arange(P), indexing="ij")
    s_all[pp.ravel(), cc.ravel(), rloc.reshape(nch, P).ravel()] = \
        ws.reshape(nch, P).ravel()
    return (idx_all, s_all, cols.reshape(nch, P).astype(np.int64),
            chunk_tile, n_round_a)


# ------------------------------------------------------------- device build

@functools.lru_cache(maxsize=2)
def _build_program(chunk_tile, n_round_a):
    nch = len(chunk_tile)
    nc = bacc.Bacc("TRN2", target_bir_lowering=False, debug=False,
                   num_devices=NCORES, num_swdge_queues=4)

    xt_in = nc.dram_tensor("xt", [BPC, KP, EP], BF, kind="ExternalInput")
    em_in = nc.dram_tensor("em", [P, nch, BPC * KP], BF, kind="ExternalInput")
    idx_in = nc.dram_tensor("idx", [P, nch * 8], I16, kind="ExternalInput")
    s_in = nc.dram_tensor("sall", [P, nch, P], BF, kind="ExternalInput")
    ident_in = nc.dram_tensor("ident", [P, P], BF, kind="ExternalInput")
    m1_in = nc.dram_tensor("m1", [KP, C], BF, kind="ExternalInput")
    m1w1_in = nc.dram_tensor("m1w1", [KP, C], BF, kind="ExternalInput")
    w0_in = [nc.dram_tensor(f"w0_{l}", [C, C], BF, kind="ExternalInput") for l in range(2)]
    w1b_in = nc.dram_tensor("w1b", [C, C], BF, kind="ExternalInput")
    mw1_in = nc.dram_tensor("mw1", [C, H], BF, kind="ExternalInput")
    mw2_in = nc.dram_tensor("mw2", [H, OPD], BF, kind="ExternalInput")
    cb_in = [nc.dram_tensor(f"cb_{l}", [C, 1], F32, kind="ExternalInput") for l in range(2)]
    mb1_in = nc.dram_tensor("mb1", [H, 1], F32, kind="ExternalInput")
    b2_in = nc.dram_tensor("b2t", [P, OPD], F32, kind="ExternalInput")
    out_ext = nc.dram_tensor("out", [BPC, N_PRED, E, PD], F32, kind="ExternalOutput")
    h4 = nc.dram_tensor("h4", [EP, BPC * C], BF)
    txp = nc.dram_tensor("txp", [EP, BPC * C], BF)

    def make_groups(c0, c1):
        return [(i, min(i + GG, c1)) for i in range(c0, c1, GG)]

    with tile.TileContext(nc) as tc:
        with tc.tile_pool(name="const", bufs=1) as cp, \
             tc.tile_pool(name="work", bufs=2) as wp, \
             tc.tile_pool(name="psA", bufs=2, space="PSUM") as psA, \
             tc.tile_pool(name="psB", bufs=2, space="PSUM") as psB:

            def const_sb(handle, shape, dtype):
                t_ = cp.tile(shape, dtype, name=handle.name + "_sb")
                nc.sync.dma_start(t_, handle.ap())
                return t_

            ident_sb = const_sb(ident_in, [P, P], BF)
            m1_sb = const_sb(m1_in, [KP, C], BF)
            m1w1_sb = const_sb(m1w1_in, [KP, C], BF)
            w0_sb = [const_sb(w0_in[l], [C, C], BF) for l in range(2)]
            w1b_sb = const_sb(w1b_in, [C, C], BF)
            mw1_sb = const_sb(mw1_in, [C, H], BF)
            mw2_sb = const_sb(mw2_in, [H, OPD], BF)
            cb_sb = [const_sb(cb_in[l], [C, 1], F32) for l in range(2)]
            mb1_sb = const_sb(mb1_in, [H, 1], F32)
            b2_sb = const_sb(b2_in, [P, OPD], F32)

            hT = cp.tile([P, BPC, EP], BF, name="hT")
            xt_ap = xt_in.ap()
            s_ap = s_in.ap()
            em_ap = em_in.ap()
            idx_ap = idx_in.ap()
            h4_ap = h4.ap()
            h4_lo = h4_ap[0:MID, :]
            txp_ap = txp.ap()

            # ---- phase A: h0^T = M1x^T @ x2^T (fused conv+embed+bias) ----
            for b in range(BPC):
                xT = wp.tile([KP, EP], BF, tag="zT", bufs=1)
                nc.sync.dma_start(xT, xt_ap[b])
                for ws in range(0, EP, 512):
                    we = min(ws + 512, EP)
                    ph = psA.tile([C, 512], F32, tag="ps_sc")
                    nc.tensor.matmul(ph[:, :we - ws], m1_sb, xT[:, ws:we],
                                     start=True, stop=True)
                    nc.scalar.copy(hT[:, b, ws:we], ph[:, :we - ws])

            # ---- scatter-round machinery ----
            def scatter_round(l, rnd, finish):
                """One round of one layer's scatter stream.

                l: 0 = x-space messages from the host em table (plain DMA),
                   1 = gather h1 rows from h4 (round A reads only h4[:MID]).
                """
                fw = KP if l == 0 else C
                elem = BPC * fw
                c0r, c1r = (0, n_round_a) if rnd == 0 else (n_round_a, nch)
                mtag = "mA" if rnd == 0 else "mB"
                stag_ = "SA" if rnd == 0 else "SB"
                ptag = "ps_e" if rnd == 0 else "ps_sc"
                ps = None
                for gi, (c0, c1) in enumerate(make_groups(c0r, c1r)):
                    G = c1 - c0
                    msgs = wp.tile([P, GG, elem], BF, tag=mtag, bufs=3)
                    if l == 0:
                        nc.sync.dma_start(msgs[:, :G, :], em_ap[:, c0:c1, :])
                    else:
                        ixs = wp.tile([P, GG * 8], I16, tag="ix" + stag_,
                                      bufs=3)
                        nc.sync.dma_start(ixs[:, :G * 8],
                                          idx_ap[:, c0 * 8:c1 * 8])
                        nc.gpsimd.dma_gather(
                            out_ap=msgs[:, :G, :],
                            in_ap=h4_lo if rnd == 0 else h4_ap,
                            idxs_ap=ixs[:, :G * 8],
                            num_idxs=G * P,
                            num_idxs_reg=G * P,
                            elem_size=elem,
                            queue_num=gi % 4,
                        )
                    S_sb = wp.tile([P, GG, P], BF, tag=stag_, bufs=3)
                    nc.sync.dma_start(S_sb[:, :G, :], s_ap[:, c0:c1, :])
                    for ci in range(c0, c1):
                        t = chunk_tile[ci]
                        first = ci == c0r or chunk_tile[ci - 1] != t
                        last = ci == c1r - 1 or chunk_tile[ci + 1] != t
                        if first:
                            ps = psA.tile([P, BPC * C], F32, tag=ptag,
                                          bufs=2 if rnd else 1)
                        k = ci - c0
                        nc.tensor.matmul(
                            ps[:, :elem], S_sb[:, k, :], msgs[:, k, :],
                            start=first, stop=last)
                        if last:
                            finish(t, ps, fw)

            # round A finisher: park partial tx1 in DRAM
            def finish_partial(t, ps, fw):
                stagA = wp.tile([P, BPC * C], BF, tag="stagA", bufs=3)
                nc.vector.tensor_copy(stagA[:, :BPC * fw], ps[:, :BPC * fw])
                nc.sync.dma_start(txp_ap[t * P:(t + 1) * P, :BPC * fw],
                                  stagA[:, :BPC * fw])

            # round B finisher: add partial, transpose, dense, relu, (h4)
            def finish_full(l, t, ps, fw):
                txl = wp.tile([P, BPC * C], BF, tag="txl", bufs=2)
                nc.sync.dma_start(txl[:, :BPC * fw],
                                  txp_ap[t * P:(t + 1) * P, :BPC * fw])
                tx1r = wp.tile([P, BPC * C], BF, tag="tx1r", bufs=2)
                nc.vector.tensor_tensor(tx1r[:, :BPC * fw], ps[:, :BPC * fw],
                                        txl[:, :BPC * fw], op=ALU.add)
                tx1T = wp.tile([C, BPC, P], BF, tag="tx1T", bufs=2)
                for b in range(BPC):
                    ptt = psB.tile([C, P], BF, tag="ps_tr", bufs=3)
                    nc.tensor.transpose(ptt[:fw, :], tx1r[:, b * fw:(b + 1) * fw],
                                        ident_sb)
                    nc.vector.tensor_copy(tx1T[:fw, b, :], ptt[:fw, :])
                wagg = m1w1_sb if l == 0 else w1b_sb
                for b in range(BPC):
                    pd = psB.tile([C, P], F32, tag="ps_d", bufs=2)
                    nc.tensor.matmul(pd, w0_sb[l], hT[:, b, t * P:(t + 1) * P],
                                     start=True, stop=False)
                    nc.tensor.matmul(pd, wagg[:fw, :], tx1T[:fw, b, :],
                                     start=False, stop=True)
                    nc.scalar.activation(hT[:, b, t * P:(t + 1) * P], pd,
                                         AF.Relu, bias=cb_sb[l], scale=1.0)
                if l == 0:
                    stag = wp.tile([P, BPC * C], BF, tag="stagA", bufs=3)
                    for b in range(BPC):
                        pt = psB.tile([P, C], BF, tag="ps_tr", bufs=3)
                        nc.tensor.transpose(pt, hT[:, b, t * P:(t + 1) * P],
                                            ident_sb)
                        nc.vector.tensor_copy(stag[:, b * C:(b + 1) * C], pt)
                    nc.sync.dma_start(h4_ap[t * P:(t + 1) * P, :], stag)

            for l in range(2):
                scatter_round(l, 0, finish_partial)
                scatter_round(l, 1, lambda t, ps, fw, l=l: finish_full(l, t, ps, fw))

            # ---- MLP + output ----
            out_ap = out_ext.ap()
            for b in range(BPC):
                zT = wp.tile([H, EP], BF, tag="zT", bufs=1)
                for ws in range(0, EP, 512):
                    we = min(ws + 512, EP)
                    pm = psA.tile([H, 512], F32, tag="ps_sc")
                    nc.tensor.matmul(pm[:, :we - ws], mw1_sb, hT[:, b, ws:we],
                                     start=True, stop=True)
                    nc.scalar.activation(zT[:, ws:we], pm[:, :we - ws],
                                         AF.Relu, bias=mb1_sb, scale=1.0)
                stagP = wp.tile([P, N_PRED, NJ, PD], BF, tag="stagP", bufs=2)
                zTb = zT.rearrange("h (q j) -> h j q", j=NJ)
                for j in range(NJ):
                    pp = psB.tile([P, OPD], F32, tag="ps_d", bufs=2)
                    nc.tensor.matmul(pp, zTb[:, j, :], mw2_sb,
                                     start=True, stop=True)
                    nc.vector.tensor_tensor(
                        out=stagP[:, :, j, :],
                        in0=pp.rearrange("p (n c) -> p n c", n=N_PRED),
                        in1=b2_sb.rearrange("p (n c) -> p n c", n=N_PRED),
                        op=ALU.add)
                out_b = out_ap[b]
                main = out_b[:, :E_MAIN, :].rearrange("n (p j) c -> p n j c", j=NJ)
                nc.gpsimd.dma_start(out=main, in_=stagP[:E_MAIN // NJ])
                tail = out_b[:, E_MAIN:E, :].rearrange("n (p j) c -> p n j c", p=1)
                nc.gpsimd.dma_start(
                    out=tail, in_=stagP[E_MAIN // NJ:E_MAIN // NJ + 1, :, :E - E_MAIN, :])

    nc.compile()
    return nc


# ----------------------------------------------------------------- kernel()

def _prep_weights(conv_w, conv_b, embed_w, embed_b,
                  cheb0_w0, cheb0_w1, cheb0_b, cheb1_w0, cheb1_w1, cheb1_b,
                  mlp_w1, mlp_b1, mlp_w2, mlp_b2):
    f32 = np.float32
    m1 = np.einsum("oit,oc->tic", conv_w.astype(f32),
                   embed_w.astype(f32)).reshape(KD, C)
    b0 = conv_b.astype(f32) @ embed_w.astype(f32) + embed_b.astype(f32)
    m1x = np.zeros((KP, C), dtype=f32)
    m1x[:KD] = m1
    m1x[KD] = b0
    shared = {
        "m1": m1x.astype(bf16),
        "m1w1": (m1x @ cheb0_w1.astype(f32)).astype(bf16),
        "w0_0": cheb0_w0.astype(bf16), "w0_1": cheb1_w0.astype(bf16),
        "w1b": cheb1_w1.astype(bf16),
        "mw1": mlp_w1.astype(bf16), "mw2": mlp_w2.astype(bf16),
        "cb_0": cheb0_b.reshape(C, 1).astype(f32),
        "cb_1": cheb1_b.reshape(C, 1).astype(f32),
        "mb1": mlp_b1.reshape(H, 1).astype(f32),
        "b2t": np.tile(mlp_b2.astype(f32).reshape(1, OPD), (P, 1)),
        "ident": np.eye(P, dtype=np.float32).astype(bf16),
    }
    return shared


def prepare(x, edge_index, conv_w, conv_b, embed_w, embed_b,
            cheb0_w0, cheb0_w1, cheb0_b, cheb1_w0, cheb1_w1, cheb1_b,
            mlp_w1, mlp_b1, mlp_w2, mlp_b2):
    """Host preprocessing: returns (compiled program, per-core in_maps)."""
    x = np.asarray(x, dtype=np.float32)
    idx_all, s_all, cols_rs, chunk_tile, n_round_a = _preprocess_graph(
        np.asarray(edge_index))

    shared = _prep_weights(
        np.asarray(conv_w, np.float32), np.asarray(conv_b, np.float32),
        np.asarray(embed_w, np.float32), np.asarray(embed_b, np.float32),
        np.asarray(cheb0_w0, np.float32), np.asarray(cheb0_w1, np.float32),
        np.asarray(cheb0_b, np.float32),
        np.asarray(cheb1_w0, np.float32), np.asarray(cheb1_w1, np.float32),
        np.asarray(cheb1_b, np.float32),
        np.asarray(mlp_w1, np.float32), np.asarray(mlp_b1, np.float32),
        np.asarray(mlp_w2, np.float32), np.asarray(mlp_b2, np.float32))
    shared.update({"idx": idx_all, "sall": s_all})

    # x: [B, T, E, D] -> [B, EP, 64] bf16: (t,i) flattened, ones col at 48
    # (carries the fused conv+embed bias), zero pad cols 49: and rows >= E.
    x2 = np.zeros((B, EP, KP), dtype=bf16)
    x2[:, :E, :KD] = x.transpose(0, 2, 1, 3).reshape(B, E, KD).astype(bf16)
    x2[:, :E, KD] = bf16(1.0)

    nc = _build_program(chunk_tile, n_round_a)

    in_maps = []
    for ci in range(NCORES):
        m = dict(shared)
        xs = x2[ci * BPC:(ci + 1) * BPC]
        m["xt"] = np.ascontiguousarray(xs.transpose(0, 2, 1))
        # layer-1 edge messages, host-gathered into chunk order:
        # em[p, c, :] = x2[:, cols[c*128+p], :] flattened over (b, k)
        xcat = np.ascontiguousarray(xs.transpose(1, 0, 2))  # [EP, BPC, KP]
        em = xcat[cols_rs]                    # [nch, P, BPC, KP]
        m["em"] = np.ascontiguousarray(
            em.transpose(1, 0, 2, 3).reshape(P, len(chunk_tile), BPC * KP))
        in_maps.append(m)
    return nc, in_maps


def kernel(**inputs):
    nc, in_maps = prepare(**inputs)
    res = run_bass_kernel_spmd(nc, in_maps, list(range(NCORES)))
    out = np.concatenate([res.results[ci]["out"] for ci in range(NCORES)],
                         axis=0)
    return np.ascontiguousarray(out, dtype=np.float32)


# revision 52
# speedup vs baseline: 1.0120x; 1.0120x over previous
"""GCN+MLP (ChebConv K=2, sym norm) Trainium2 Bass/Tile kernel.

nn_GCNMLP_81320910782821: out = MLP(relu(cheb1(relu(cheb0(embed(conv(x)))))))
with cheb(h) = h@W0 + (S@h)@W1 + b, S = -D^-1/2 A D^-1/2 (160k random edges,
E=10000 nodes, C=128 channels).

Sharding: data-parallel over batch B=32 -> 8 NeuronCores x 4 batch elems.
Per-core device kernel (bf16 compute, fp32 PSUM accumulation):
  * h kept SBUF-resident transposed: hT [C=128 part, 4b, E] bf16.
  * The segment-sum is one-hot matmul scatter over 128-edge chunks grouped
    by 128-row dest tiles; the one-hot matrices S_chunk[e,d] = w_e*(row_e==d)
    are host-precomputed and streamed from DRAM.
  * Layer 1 aggregates in x-space ((S@h0)@W1 == (S@X2)@(M1@W1)), so its
    messages are a pure input-layout transform that the host pre-gathers
    into edge order (em table) -> no device gather at all in layer 1.
  * Layer 2 messages are device-gathered from a row-major copy of h1 (h4,
    [E, 4b*C] bf16, 1024B rows) via dma_gather across 4 SWDGE queues.
  * Each dest tile's edges are split into two rounds by source node
    (< / >= MID): round-A of layer 2 only reads h4[:MID], so its gathers and
    scatters overlap the back half of layer 1; round partial sums round-trip
    through a DRAM buffer and are re-added on DVE.
  * Output uses the e = 79*p + j partition mapping so the final
    [N_PRED, E, PD] DMA has 1264B contiguous runs.

kernel(**inputs) takes FULL unsharded fp32/int64 inputs and returns the FULL
[B, N_PRED, E, PD] fp32 output. The Bass program is input-shape static but
depends on the per-tile chunk counts of the actual graph; it is built and
compiled on first call (cached per chunk signature).
"""

import functools

import numpy as np
import ml_dtypes

import concourse.bacc as bacc
import concourse.bass as bass
import concourse.mybir as mybir
import concourse.tile as tile
from concourse.bass_utils import run_bass_kernel_spmd

B, T, E, D = 32, 12, 10000, 4
C, H = 128, 64
N_PRED, PD = 12, 4
NCORES = 8
BPC = B // NCORES          # batch elems per core
P = 128
NJ = 79                    # e = NJ*p + j partition mapping
EP = P * NJ                # 10112 (E padded)
NT = EP // P               # 79 dest tiles of 128 rows
KD = T * D                 # 48 contraction dim of fused conv+embed
KDX = KD + 1               # + ones column carrying the fused bias
KP = 64                    # KDX padded (layer-1 message row 4*64*2B = 512B)
GG = 8                     # chunks per message group
OPD = N_PRED * PD          # 48
E_MAIN = (E // NJ) * NJ    # 9954 = 126*79 (rows covered by partitions 0..125)
MID = 40 * P               # 5120: source split point (h4-tile aligned)

BF = mybir.dt.bfloat16
F32 = mybir.dt.float32
I16 = mybir.dt.int16
AF = mybir.ActivationFunctionType
ALU = mybir.AluOpType
bf16 = ml_dtypes.bfloat16


# ---------------------------------------------------------------- host side

def _preprocess_graph(edge_index):
    """Sort edges by (dest tile, source round, dest row); pack into 128-edge
    chunks. Round A = sources < MID, round B = rest; within each round the
    chunk stream is dest-tile contiguous.

    Returns (idx_all [128, nch*8] i16, s_all [128, nch, 128] bf16,
             cols_rs [nch, 128] i64, chunk_tile tuple, n_round_a int).
    """
    row = np.asarray(edge_index[0], dtype=np.int64)
    col = np.asarray(edge_index[1], dtype=np.int64)
    deg = np.bincount(row, minlength=E).astype(np.float32)
    dis = np.where(deg > 0, 1.0 / np.sqrt(np.maximum(deg, 1.0)), 0.0).astype(np.float32)
    w = (-dis[row] * dis[col]).astype(np.float32)
    order = np.argsort(row, kind="stable")
    r_s, c_s, w_s = row[order], col[order], w[order]
    bounds = np.searchsorted(r_s, np.arange(NT + 1) * P)

    streams = {0: ([], [], [], []), 1: ([], [], [], [])}  # cols, rloc, ws, tile
    for t in range(NT):
        s, e_ = int(bounds[t]), int(bounds[t + 1])
        tc, tr, tw = c_s[s:e_], r_s[s:e_], w_s[s:e_]
        early = tc < MID
        for rnd, mask in ((0, early), (1, ~early)):
            cc, rr, ww = tc[mask], tr[mask], tw[mask]
            n = len(cc)
            nch_t = max(1, -(-n // P))
            pad = nch_t * P - n
            cols_p, rloc_p, ws_p, tl = streams[rnd]
            cols_p.append(np.pad(cc, (0, pad)))
            rloc_p.append(np.pad(rr - t * P, (0, pad)))
            ws_p.append(np.pad(ww, (0, pad)))
            tl += [t] * nch_t

    cols = np.concatenate(streams[0][0] + streams[1][0]).astype(np.int16)
    rloc = np.concatenate(streams[0][1] + streams[1][1]).astype(np.int64)
    ws = np.concatenate(streams[0][2] + streams[1][2]).astype(np.float32)
    chunk_tile = tuple(streams[0][3]) + tuple(streams[1][3])
    n_round_a = len(streams[0][3])
    nch = len(chunk_tile)
    # dma_gather index layout: idx i at [partition i%16, col i//16], x8 replicas
    idx_all = np.tile(cols.reshape(nch * 8, 16).T, (8, 1)).astype(np.int16)
    # one-hot scatter matrices, laid out [p(edge-in-chunk), chunk, dest]
    s_all = np.zeros((P, nch, P), dtype=bf16)
    cc, pp = np.meshgrid(np.arange(nch), np.arange(P), indexing="ij")
    s_all[pp.ravel(), cc.ravel(), rloc.reshape(nch, P).ravel()] = \
        ws.reshape(nch, P).ravel()
    return (idx_all, s_all, cols.reshape(nch, P).astype(np.int64),
            chunk_tile, n_round_a)


# ------------------------------------------------------------- device build

@functools.lru_cache(maxsize=2)
def _build_program(chunk_tile, n_round_a):
    nch = len(chunk_tile)
    nc = bacc.Bacc("TRN2", target_bir_lowering=False, debug=False,
                   num_devices=NCORES, num_swdge_queues=4)

    xt_in = nc.dram_tensor("xt", [BPC, KP, EP], BF, kind="ExternalInput")
    em_in = nc.dram_tensor("em", [P, nch, BPC * KP], BF, kind="ExternalInput")
    idx_in = nc.dram_tensor("idx", [P, nch * 8], I16, kind="ExternalInput")
    s_in = nc.dram_tensor("sall", [P, nch, P], BF, kind="ExternalInput")
    ident_in = nc.dram_tensor("ident", [P, P], BF, kind="ExternalInput")
    m1_in = nc.dram_tensor("m1", [KP, C], BF, kind="ExternalInput")
    m1w1_in = nc.dram_tensor("m1w1", [KP, C], BF, kind="ExternalInput")
    w0_in = [nc.dram_tensor(f"w0_{l}", [C, C], BF, kind="ExternalInput") for l in range(2)]
    w1b_in = nc.dram_tensor("w1b", [C, C], BF, kind="ExternalInput")
    mw1_in = nc.dram_tensor("mw1", [C, H], BF, kind="ExternalInput")
    mw2_in = nc.dram_tensor("mw2", [H, OPD], BF, kind="ExternalInput")
    cb_in = [nc.dram_tensor(f"cb_{l}", [C, 1], F32, kind="ExternalInput") for l in range(2)]
    mb1_in = nc.dram_tensor("mb1", [H, 1], F32, kind="ExternalInput")
    b2_in = nc.dram_tensor("b2t", [P, OPD], F32, kind="ExternalInput")
    out_ext = nc.dram_tensor("out", [BPC, N_PRED, E, PD], F32, kind="ExternalOutput")
    h4 = nc.dram_tensor("h4", [EP, BPC * C], BF)
    txp = nc.dram_tensor("txp", [EP, BPC * C], BF)

    def make_groups(c0, c1):
        return [(i, min(i + GG, c1)) for i in range(c0, c1, GG)]

    with tile.TileContext(nc) as tc:
        with tc.tile_pool(name="const", bufs=1) as cp, \
             tc.tile_pool(name="work", bufs=2) as wp, \
             tc.tile_pool(name="psA", bufs=2, space="PSUM") as psA, \
             tc.tile_pool(name="psB", bufs=2, space="PSUM") as psB:

            def const_sb(handle, shape, dtype):
                t_ = cp.tile(shape, dtype, name=handle.name + "_sb")
                nc.sync.dma_start(t_, handle.ap())
                return t_

            ident_sb = const_sb(ident_in, [P, P], BF)
            m1_sb = const_sb(m1_in, [KP, C], BF)
            m1w1_sb = const_sb(m1w1_in, [KP, C], BF)
            w0_sb = [const_sb(w0_in[l], [C, C], BF) for l in range(2)]
            w1b_sb = const_sb(w1b_in, [C, C], BF)
            mw1_sb = const_sb(mw1_in, [C, H], BF)
            mw2_sb = const_sb(mw2_in, [H, OPD], BF)
            cb_sb = [const_sb(cb_in[l], [C, 1], F32) for l in range(2)]
            mb1_sb = const_sb(mb1_in, [H, 1], F32)
            b2_sb = const_sb(b2_in, [P, OPD], F32)

            hT = cp.tile([P, BPC, EP], BF, name="hT")
            xt_ap = xt_in.ap()
            s_ap = s_in.ap()
            em_ap = em_in.ap()
            idx_ap = idx_in.ap()
            h4_ap = h4.ap()
            h4_lo = h4_ap[0:MID, :]
            txp_ap = txp.ap()

            # ---- phase A: h0^T = M1x^T @ x2^T (fused conv+embed+bias) ----
            for b in range(BPC):
                xT = wp.tile([KP, EP], BF, tag="zT", bufs=1)
                nc.sync.dma_start(xT, xt_ap[b])
                for ws in range(0, EP, 512):
                    we = min(ws + 512, EP)
                    ph = psA.tile([C, 512], F32, tag="ps_sc")
                    nc.tensor.matmul(ph[:, :we - ws], m1_sb, xT[:, ws:we],
                                     start=True, stop=True)
                    nc.scalar.copy(hT[:, b, ws:we], ph[:, :we - ws])

            # ---- scatter-round machinery ----
            def scatter_round(l, rnd, finish):
                """One round of one layer's scatter stream.

                l: 0 = x-space messages from the host em table (plain DMA),
                   1 = gather h1 rows from h4 (round A reads only h4[:MID]).
                """
                fw = KP if l == 0 else C
                elem = BPC * fw
                c0r, c1r = (0, n_round_a) if rnd == 0 else (n_round_a, nch)
                mtag = "mA" if rnd == 0 else "mB"
                stag_ = "SA" if rnd == 0 else "SB"
                ptag = "ps_e" if rnd == 0 else "ps_sc"
                ps = None
                for gi, (c0, c1) in enumerate(make_groups(c0r, c1r)):
                    G = c1 - c0
                    msgs = wp.tile([P, GG, elem], BF, tag=mtag, bufs=3)
                    if l == 0:
                        nc.sync.dma_start(msgs[:, :G, :], em_ap[:, c0:c1, :])
                    else:
                        ixs = wp.tile([P, GG * 8], I16, tag="ix" + stag_,
                                      bufs=3)
                        nc.sync.dma_start(ixs[:, :G * 8],
                                          idx_ap[:, c0 * 8:c1 * 8])
                        nc.gpsimd.dma_gather(
                            out_ap=msgs[:, :G, :],
                            in_ap=h4_lo if rnd == 0 else h4_ap,
                            idxs_ap=ixs[:, :G * 8],
                            num_idxs=G * P,
                            num_idxs_reg=G * P,
                            elem_size=elem,
                            queue_num=gi % 4,
                        )
                    S_sb = wp.tile([P, GG, P], BF, tag=stag_, bufs=3)
                    nc.sync.dma_start(S_sb[:, :G, :], s_ap[:, c0:c1, :])
                    for ci in range(c0, c1):
                        t = chunk_tile[ci]
                        first = ci == c0r or chunk_tile[ci - 1] != t
                        last = ci == c1r - 1 or chunk_tile[ci + 1] != t
                        if first:
                            ps = psA.tile([P, BPC * C], F32, tag=ptag, bufs=2)
                        k = ci - c0
                        nc.tensor.matmul(
                            ps[:, :elem], S_sb[:, k, :], msgs[:, k, :],
                            start=first, stop=last)
                        if last:
                            finish(t, ps, fw)

            # round A finisher: park partial tx1 in DRAM
            def finish_partial(t, ps, fw):
                stagA = wp.tile([P, BPC * C], BF, tag="stagA", bufs=3)
                nc.vector.tensor_copy(stagA[:, :BPC * fw], ps[:, :BPC * fw])
                nc.sync.dma_start(txp_ap[t * P:(t + 1) * P, :BPC * fw],
                                  stagA[:, :BPC * fw])

            # round B finisher: add partial, transpose, dense, relu, (h4)
            def finish_full(l, t, ps, fw):
                txl = wp.tile([P, BPC * C], BF, tag="txl", bufs=2)
                nc.sync.dma_start(txl[:, :BPC * fw],
                                  txp_ap[t * P:(t + 1) * P, :BPC * fw])
                tx1r = wp.tile([P, BPC * C], BF, tag="tx1r", bufs=2)
                nc.vector.tensor_tensor(tx1r[:, :BPC * fw], ps[:, :BPC * fw],
                                        txl[:, :BPC * fw], op=ALU.add)
                tx1T = wp.tile([C, BPC, P], BF, tag="tx1T", bufs=2)
                for b in range(BPC):
                    ptt = psB.tile([C, P], BF, tag="ps_tr", bufs=2)
                    nc.tensor.transpose(ptt[:fw, :], tx1r[:, b * fw:(b + 1) * fw],
                                        ident_sb)
                    nc.vector.tensor_copy(tx1T[:fw, b, :], ptt[:fw, :])
                wagg = m1w1_sb if l == 0 else w1b_sb
                for b in range(BPC):
                    pd = psB.tile([C, P], F32, tag="ps_d", bufs=2)
                    nc.tensor.matmul(pd, w0_sb[l], hT[:, b, t * P:(t + 1) * P],
                                     start=True, stop=False)
                    nc.tensor.matmul(pd, wagg[:fw, :], tx1T[:fw, b, :],
                                     start=False, stop=True)
                    nc.scalar.activation(hT[:, b, t * P:(t + 1) * P], pd,
                                         AF.Relu, bias=cb_sb[l], scale=1.0)
                if l == 0:
                    stag = wp.tile([P, BPC * C], BF, tag="stagA", bufs=3)
                    for b in range(BPC):
                        pt = psB.tile([P, C], BF, tag="ps_tr", bufs=2)
                        nc.tensor.transpose(pt, hT[:, b, t * P:(t + 1) * P],
                                            ident_sb)
                        nc.vector.tensor_copy(stag[:, b * C:(b + 1) * C], pt)
                    nc.sync.dma_start(h4_ap[t * P:(t + 1) * P, :], stag)

            for l in range(2):
                scatter_round(l, 0, finish_partial)
                scatter_round(l, 1, lambda t, ps, fw, l=l: finish_full(l, t, ps, fw))

            # ---- MLP + output ----
            out_ap = out_ext.ap()
            for b in range(BPC):
                zT = wp.tile([H, EP], BF, tag="zT", bufs=1)
                for ws in range(0, EP, 512):
                    we = min(ws + 512, EP)
                    pm = psA.tile([H, 512], F32, tag="ps_sc")
                    nc.tensor.matmul(pm[:, :we - ws], mw1_sb, hT[:, b, ws:we],
                                     start=True, stop=True)
                    nc.scalar.activation(zT[:, ws:we], pm[:, :we - ws],
                                         AF.Relu, bias=mb1_sb, scale=1.0)
                stagP = wp.tile([P, N_PRED, NJ, PD], BF, tag="stagP", bufs=2)
                zTb = zT.rearrange("h (q j) -> h j q", j=NJ)
                for j in range(NJ):
                    pp = psB.tile([P, OPD], F32, tag="ps_d", bufs=2)
                    nc.tensor.matmul(pp, zTb[:, j, :], mw2_sb,
                                     start=True, stop=True)
                    nc.vector.tensor_tensor(
                        out=stagP[:, :, j, :],
                        in0=pp.rearrange("p (n c) -> p n c", n=N_PRED),
                        in1=b2_sb.rearrange("p (n c) -> p n c", n=N_PRED),
                        op=ALU.add)
                out_b = out_ap[b]
                main = out_b[:, :E_MAIN, :].rearrange("n (p j) c -> p n j c", j=NJ)
                nc.gpsimd.dma_start(out=main, in_=stagP[:E_MAIN // NJ])
                tail = out_b[:, E_MAIN:E, :].rearrange("n (p j) c -> p n j c", p=1)
                nc.gpsimd.dma_start(
                    out=tail, in_=stagP[E_MAIN // NJ:E_MAIN // NJ + 1, :, :E - E_MAIN, :])

    nc.compile()
    return nc


# ----------------------------------------------------------------- kernel()

def _prep_weights(conv_w, conv_b, embed_w, embed_b,
                  cheb0_w0, cheb0_w1, cheb0_b, cheb1_w0, cheb1_w1, cheb1_b,
                  mlp_w1, mlp_b1, mlp_w2, mlp_b2):
    f32 = np.float32
    m1 = np.einsum("oit,oc->tic", conv_w.astype(f32),
                   embed_w.astype(f32)).reshape(KD, C)
    b0 = conv_b.astype(f32) @ embed_w.astype(f32) + embed_b.astype(f32)
    m1x = np.zeros((KP, C), dtype=f32)
    m1x[:KD] = m1
    m1x[KD] = b0
    shared = {
        "m1": m1x.astype(bf16),
        "m1w1": (m1x @ cheb0_w1.astype(f32)).astype(bf16),
        "w0_0": cheb0_w0.astype(bf16), "w0_1": cheb1_w0.astype(bf16),
        "w1b": cheb1_w1.astype(bf16),
        "mw1": mlp_w1.astype(bf16), "mw2": mlp_w2.astype(bf16),
        "cb_0": cheb0_b.reshape(C, 1).astype(f32),
        "cb_1": cheb1_b.reshape(C, 1).astype(f32),
        "mb1": mlp_b1.reshape(H, 1).astype(f32),
        "b2t": np.tile(mlp_b2.astype(f32).reshape(1, OPD), (P, 1)),
        "ident": np.eye(P, dtype=np.float32).astype(bf16),
    }
    return shared


def prepare(x, edge_index, conv_w, conv_b, embed_w, embed_b,
            cheb0_w0, cheb0_w1, cheb0_b, cheb1_w0, cheb1_w1, cheb1_b,
            mlp_w1, mlp_b1, mlp_w2, mlp_b2):
    """Host preprocessing: returns (compiled program, per-core in_maps)."""
    x = np.asarray(x, dtype=np.float32)
    idx_all, s_all, cols_rs, chunk_tile, n_round_a = _preprocess_graph(
        np.asarray(edge_index))

    shared = _prep_weights(
        np.asarray(conv_w, np.float32), np.asarray(conv_b, np.float32),
        np.asarray(embed_w, np.float32), np.asarray(embed_b, np.float32),
        np.asarray(cheb0_w0, np.float32), np.asarray(cheb0_w1, np.float32),
        np.asarray(cheb0_b, np.float32),
        np.asarray(cheb1_w0, np.float32), np.asarray(cheb1_w1, np.float32),
        np.asarray(cheb1_b, np.float32),
        np.asarray(mlp_w1, np.float32), np.asarray(mlp_b1, np.float32),
        np.asarray(mlp_w2, np.float32), np.asarray(mlp_b2, np.float32))
    shared.update({"idx": idx_all, "sall": s_all})

    # x: [B, T, E, D] -> [B, EP, 64] bf16: (t,i) flattened, ones col at 48
    # (carries the fused conv+embed bias), zero pad cols 49: and rows >= E.
    x2 = np.zeros((B, EP, KP), dtype=bf16)
    x2[:, :E, :KD] = x.transpose(0, 2, 1, 3).reshape(B, E, KD).astype(bf16)
    x2[:, :E, KD] = bf16(1.0)

    nc = _build_program(chunk_tile, n_round_a)

    in_maps = []
    for ci in range(NCORES):
        m = dict(shared)
        xs = x2[ci * BPC:(ci + 1) * BPC]
        m["xt"] = np.ascontiguousarray(xs.transpose(0, 2, 1))
        # layer-1 edge messages, host-gathered into chunk order:
        # em[p, c, :] = x2[:, cols[c*128+p], :] flattened over (b, k)
        xcat = np.ascontiguousarray(xs.transpose(1, 0, 2))  # [EP, BPC, KP]
        em = xcat[cols_rs]                    # [nch, P, BPC, KP]
        m["em"] = np.ascontiguousarray(
            em.transpose(1, 0, 2, 3).reshape(P, len(chunk_tile), BPC * KP))
        in_maps.append(m)
    return nc, in_maps


def kernel(**inputs):
    nc, in_maps = prepare(**inputs)
    res = run_bass_kernel_spmd(nc, in_maps, list(range(NCORES)))
    out = np.concatenate([res.results[ci]["out"] for ci in range(NCORES)],
                         axis=0)
    return np.ascontiguousarray(out, dtype=np.float32)


# revision 56
# speedup vs baseline: 1.0589x; 1.0464x over previous
"""GCN+MLP (ChebConv K=2, sym norm) Trainium2 Bass/Tile kernel.

nn_GCNMLP_81320910782821: out = MLP(relu(cheb1(relu(cheb0(embed(conv(x)))))))
with cheb(h) = h@W0 + (S@h)@W1 + b, S = -D^-1/2 A D^-1/2 (160k random edges,
E=10000 nodes, C=128 channels).

Sharding: data-parallel over batch B=32 -> 8 NeuronCores x 4 batch elems.
Per-core device kernel (bf16 compute, fp32 PSUM accumulation):
  * h kept SBUF-resident transposed: hT [C=128 part, 4b, E] bf16.
  * The segment-sum is one-hot matmul scatter over 128-edge chunks grouped
    by 128-row dest tiles; the one-hot matrices S_chunk[e,d] = w_e*(row_e==d)
    are host-precomputed and streamed from DRAM.
  * Layer 1 aggregates in x-space ((S@h0)@W1 == (S@X2)@(M1@W1)), so its
    messages are a pure input-layout transform that the host pre-gathers
    into edge order (em table) -> no device gather at all in layer 1.
  * Layer 2 messages are device-gathered from a row-major copy of h1 (h4,
    [E, 4b*C] bf16, 1024B rows) via dma_gather across 4 SWDGE queues.
  * Each dest tile's edges are split into two rounds by source node
    (< / >= MID): round-A of layer 2 only reads h4[:MID], so its gathers and
    scatters overlap the back half of layer 1; round partial sums round-trip
    through a DRAM buffer and are re-added on DVE.
  * Output uses the e = 79*p + j partition mapping so the final
    [N_PRED, E, PD] DMA has 1264B contiguous runs.

kernel(**inputs) takes FULL unsharded fp32/int64 inputs and returns the FULL
[B, N_PRED, E, PD] fp32 output. The Bass program is input-shape static but
depends on the per-tile chunk counts of the actual graph; it is built and
compiled on first call (cached per chunk signature).
"""

import functools

import numpy as np
import ml_dtypes

import concourse.bacc as bacc
import concourse.bass as bass
import concourse.mybir as mybir
import concourse.tile as tile
from concourse.bass_utils import run_bass_kernel_spmd

B, T, E, D = 32, 12, 10000, 4
C, H = 128, 64
N_PRED, PD = 12, 4
NCORES = 8
BPC = B // NCORES          # batch elems per core
P = 128
NJ = 79                    # e = NJ*p + j partition mapping
EP = P * NJ                # 10112 (E padded)
NT = EP // P               # 79 dest tiles of 128 rows
KD = T * D                 # 48 contraction dim of fused conv+embed
KDX = KD + 1               # + ones column carrying the fused bias
KP = 64                    # KDX padded (layer-1 message row 4*64*2B = 512B)
GG = 8                     # chunks per message group
OPD = N_PRED * PD          # 48
E_MAIN = (E // NJ) * NJ    # 9954 = 126*79 (rows covered by partitions 0..125)
MID = 40 * P               # 5120: source split point (h4-tile aligned)

BF = mybir.dt.bfloat16
F32 = mybir.dt.float32
I16 = mybir.dt.int16
AF = mybir.ActivationFunctionType
ALU = mybir.AluOpType
bf16 = ml_dtypes.bfloat16


# ---------------------------------------------------------------- host side

def _preprocess_graph(edge_index):
    """Sort edges by (dest tile, source round, dest row); pack into 128-edge
    chunks. Round A = sources < MID, round B = rest; within each round the
    chunk stream is dest-tile contiguous.

    Returns (idx_all [128, nch*8] i16, s_all [128, nch, 128] bf16,
             cols_rs [nch, 128] i64, chunk_tile tuple, n_round_a int).
    """
    row = np.asarray(edge_index[0], dtype=np.int64)
    col = np.asarray(edge_index[1], dtype=np.int64)
    deg = np.bincount(row, minlength=E).astype(np.float32)
    dis = np.where(deg > 0, 1.0 / np.sqrt(np.maximum(deg, 1.0)), 0.0).astype(np.float32)
    w = (-dis[row] * dis[col]).astype(np.float32)
    order = np.argsort(row, kind="stable")
    r_s, c_s, w_s = row[order], col[order], w[order]
    bounds = np.searchsorted(r_s, np.arange(NT + 1) * P)

    cols_p, rloc_p, ws_p, tl = [], [], [], []
    for t in range(NT):
        s, e_ = int(bounds[t]), int(bounds[t + 1])
        tc, tr, tw = c_s[s:e_], r_s[s:e_], w_s[s:e_]
        n = len(tc)
        nch_t = max(1, -(-n // P))
        pad = nch_t * P - n
        cols_p.append(np.pad(tc, (0, pad)))
        rloc_p.append(np.pad(tr - t * P, (0, pad)))
        ws_p.append(np.pad(tw, (0, pad)))
        tl += [t] * nch_t

    cols = np.concatenate(cols_p).astype(np.int16)
    rloc = np.concatenate(rloc_p).astype(np.int64)
    ws = np.concatenate(ws_p).astype(np.float32)
    chunk_tile = tuple(tl)
    n_round_a = 0
    nch = len(chunk_tile)
    # dma_gather index layout: idx i at [partition i%16, col i//16], x8 replicas
    idx_all = np.tile(cols.reshape(nch * 8, 16).T, (8, 1)).astype(np.int16)
    # one-hot scatter matrices, laid out [p(edge-in-chunk), chunk, dest]
    s_all = np.zeros((P, nch, P), dtype=bf16)
    cc, pp = np.meshgrid(np.arange(nch), np.arange(P), indexing="ij")
    s_all[pp.ravel(), cc.ravel(), rloc.reshape(nch, P).ravel()] = \
        ws.reshape(nch, P).ravel()
    return (idx_all, s_all, cols.reshape(nch, P).astype(np.int64),
            chunk_tile, n_round_a)


# ------------------------------------------------------------- device build

@functools.lru_cache(maxsize=2)
def _build_program(chunk_tile, n_round_a):
    nch = len(chunk_tile)
    nc = bacc.Bacc("TRN2", target_bir_lowering=False, debug=False,
                   num_devices=NCORES, num_swdge_queues=4)

    xt_in = nc.dram_tensor("xt", [BPC, KP, EP], BF, kind="ExternalInput")
    em_in = nc.dram_tensor("em", [P, nch, BPC * KP], BF, kind="ExternalInput")
    idx_in = nc.dram_tensor("idx", [P, nch * 8], I16, kind="ExternalInput")
    s_in = nc.dram_tensor("sall", [P, nch, P], BF, kind="ExternalInput")
    ident_in = nc.dram_tensor("ident", [P, P], BF, kind="ExternalInput")
    m1_in = nc.dram_tensor("m1", [KP, C], BF, kind="ExternalInput")
    m1w1_in = nc.dram_tensor("m1w1", [KP, C], BF, kind="ExternalInput")
    w0_in = [nc.dram_tensor(f"w0_{l}", [C, C], BF, kind="ExternalInput") for l in range(2)]
    w1b_in = nc.dram_tensor("w1b", [C, C], BF, kind="ExternalInput")
    mw1_in = nc.dram_tensor("mw1", [C, H], BF, kind="ExternalInput")
    mw2_in = nc.dram_tensor("mw2", [H, OPD], BF, kind="ExternalInput")
    cb_in = [nc.dram_tensor(f"cb_{l}", [C, 1], F32, kind="ExternalInput") for l in range(2)]
    mb1_in = nc.dram_tensor("mb1", [H, 1], F32, kind="ExternalInput")
    b2_in = nc.dram_tensor("b2t", [P, OPD], F32, kind="ExternalInput")
    out_ext = nc.dram_tensor("out", [BPC, N_PRED, E, PD], F32, kind="ExternalOutput")
    h4 = nc.dram_tensor("h4", [EP, BPC * C], BF)

    def make_groups(c0, c1):
        return [(i, min(i + GG, c1)) for i in range(c0, c1, GG)]

    with tile.TileContext(nc) as tc:
        with tc.tile_pool(name="const", bufs=1) as cp, \
             tc.tile_pool(name="work", bufs=2) as wp, \
             tc.tile_pool(name="psA", bufs=2, space="PSUM") as psA, \
             tc.tile_pool(name="psB", bufs=2, space="PSUM") as psB:

            def const_sb(handle, shape, dtype):
                t_ = cp.tile(shape, dtype, name=handle.name + "_sb")
                nc.sync.dma_start(t_, handle.ap())
                return t_

            ident_sb = const_sb(ident_in, [P, P], BF)
            m1_sb = const_sb(m1_in, [KP, C], BF)
            m1w1_sb = const_sb(m1w1_in, [KP, C], BF)
            w0_sb = [const_sb(w0_in[l], [C, C], BF) for l in range(2)]
            w1b_sb = const_sb(w1b_in, [C, C], BF)
            mw1_sb = const_sb(mw1_in, [C, H], BF)
            mw2_sb = const_sb(mw2_in, [H, OPD], BF)
            cb_sb = [const_sb(cb_in[l], [C, 1], F32) for l in range(2)]
            mb1_sb = const_sb(mb1_in, [H, 1], F32)
            b2_sb = const_sb(b2_in, [P, OPD], F32)

            hT = cp.tile([P, BPC, EP], BF, name="hT")
            xt_ap = xt_in.ap()
            s_ap = s_in.ap()
            em_ap = em_in.ap()
            idx_ap = idx_in.ap()
            h4_ap = h4.ap()

            # ---- phase A: h0^T = M1x^T @ x2^T (fused conv+embed+bias) ----
            for b in range(BPC):
                xT = wp.tile([KP, EP], BF, tag="zT", bufs=1)
                nc.sync.dma_start(xT, xt_ap[b])
                for ws in range(0, EP, 512):
                    we = min(ws + 512, EP)
                    ph = psA.tile([C, 512], F32, tag="ps_sc", bufs=3)
                    nc.tensor.matmul(ph[:, :we - ws], m1_sb, xT[:, ws:we],
                                     start=True, stop=True)
                    nc.scalar.copy(hT[:, b, ws:we], ph[:, :we - ws])

            # ---- scatter machinery (one tile-major round per layer) ----
            def scatter_round(l, finish):
                """One layer's scatter stream.

                l: 0 = x-space messages from the host em table (plain DMA),
                   1 = gather h1 rows from h4.
                """
                fw = KP if l == 0 else C
                elem = BPC * fw
                ps = None
                for gi, (c0, c1) in enumerate(make_groups(0, nch)):
                    G = c1 - c0
                    msgs = wp.tile([P, GG, elem], BF, tag="mB", bufs=4)
                    if l == 0:
                        nc.sync.dma_start(msgs[:, :G, :], em_ap[:, c0:c1, :])
                    else:
                        ixs = wp.tile([P, GG * 8], I16, tag="ix", bufs=3)
                        nc.sync.dma_start(ixs[:, :G * 8],
                                          idx_ap[:, c0 * 8:c1 * 8])
                        nc.gpsimd.dma_gather(
                            out_ap=msgs[:, :G, :],
                            in_ap=h4_ap,
                            idxs_ap=ixs[:, :G * 8],
                            num_idxs=G * P,
                            num_idxs_reg=G * P,
                            elem_size=elem,
                            queue_num=gi % 4,
                        )
                    S_sb = wp.tile([P, GG, P], BF, tag="SB", bufs=4)
                    nc.sync.dma_start(S_sb[:, :G, :], s_ap[:, c0:c1, :])
                    for ci in range(c0, c1):
                        t = chunk_tile[ci]
                        first = ci == 0 or chunk_tile[ci - 1] != t
                        last = ci == nch - 1 or chunk_tile[ci + 1] != t
                        if first:
                            ps = psA.tile([P, BPC * C], F32, tag="ps_sc",
                                          bufs=3)
                        k = ci - c0
                        nc.tensor.matmul(
                            ps[:, :elem], S_sb[:, k, :], msgs[:, k, :],
                            start=first, stop=last)
                        if last:
                            finish(t, ps, fw)

            # finisher: evac, transpose, dense, relu, (h4 table for l=0)
            def finish_full(l, t, ps, fw):
                tx1r = wp.tile([P, BPC * C], BF, tag="tx1r", bufs=2)
                nc.vector.tensor_copy(tx1r[:, :BPC * fw], ps[:, :BPC * fw])
                tx1T = wp.tile([C, BPC, P], BF, tag="tx1T", bufs=2)
                for b in range(BPC):
                    ptt = psB.tile([C, P], BF, tag="ps_tr", bufs=2)
                    nc.tensor.transpose(ptt[:fw, :], tx1r[:, b * fw:(b + 1) * fw],
                                        ident_sb)
                    nc.vector.tensor_copy(tx1T[:fw, b, :], ptt[:fw, :])
                wagg = m1w1_sb if l == 0 else w1b_sb
                for b in range(BPC):
                    pd = psB.tile([C, P], F32, tag="ps_d", bufs=2)
                    nc.tensor.matmul(pd, w0_sb[l], hT[:, b, t * P:(t + 1) * P],
                                     start=True, stop=False)
                    nc.tensor.matmul(pd, wagg[:fw, :], tx1T[:fw, b, :],
                                     start=False, stop=True)
                    nc.scalar.activation(hT[:, b, t * P:(t + 1) * P], pd,
                                         AF.Relu, bias=cb_sb[l], scale=1.0)
                if l == 0:
                    stag = wp.tile([P, BPC * C], BF, tag="stagA", bufs=3)
                    for b in range(BPC):
                        pt = psB.tile([P, C], BF, tag="ps_tr", bufs=2)
                        nc.tensor.transpose(pt, hT[:, b, t * P:(t + 1) * P],
                                            ident_sb)
                        nc.vector.tensor_copy(stag[:, b * C:(b + 1) * C], pt)
                    nc.sync.dma_start(h4_ap[t * P:(t + 1) * P, :], stag)

            for l in range(2):
                scatter_round(l, lambda t, ps, fw, l=l: finish_full(l, t, ps, fw))

            # ---- MLP + output ----
            out_ap = out_ext.ap()
            for b in range(BPC):
                zT = wp.tile([H, EP], BF, tag="zT", bufs=1)
                for ws in range(0, EP, 512):
                    we = min(ws + 512, EP)
                    pm = psA.tile([H, 512], F32, tag="ps_sc", bufs=3)
                    nc.tensor.matmul(pm[:, :we - ws], mw1_sb, hT[:, b, ws:we],
                                     start=True, stop=True)
                    nc.scalar.activation(zT[:, ws:we], pm[:, :we - ws],
                                         AF.Relu, bias=mb1_sb, scale=1.0)
                stagP = wp.tile([P, N_PRED, NJ, PD], BF, tag="stagP", bufs=2)
                zTb = zT.rearrange("h (q j) -> h j q", j=NJ)
                for j in range(NJ):
                    pp = psB.tile([P, OPD], F32, tag="ps_d", bufs=2)
                    nc.tensor.matmul(pp, zTb[:, j, :], mw2_sb,
                                     start=True, stop=True)
                    nc.vector.tensor_tensor(
                        out=stagP[:, :, j, :],
                        in0=pp.rearrange("p (n c) -> p n c", n=N_PRED),
                        in1=b2_sb.rearrange("p (n c) -> p n c", n=N_PRED),
                        op=ALU.add)
                out_b = out_ap[b]
                main = out_b[:, :E_MAIN, :].rearrange("n (p j) c -> p n j c", j=NJ)
                nc.gpsimd.dma_start(out=main, in_=stagP[:E_MAIN // NJ])
                tail = out_b[:, E_MAIN:E, :].rearrange("n (p j) c -> p n j c", p=1)
                nc.gpsimd.dma_start(
                    out=tail, in_=stagP[E_MAIN // NJ:E_MAIN // NJ + 1, :, :E - E_MAIN, :])

    nc.compile()
    return nc


# ----------------------------------------------------------------- kernel()

def _prep_weights(conv_w, conv_b, embed_w, embed_b,
                  cheb0_w0, cheb0_w1, cheb0_b, cheb1_w0, cheb1_w1, cheb1_b,
                  mlp_w1, mlp_b1, mlp_w2, mlp_b2):
    f32 = np.float32
    m1 = np.einsum("oit,oc->tic", conv_w.astype(f32),
                   embed_w.astype(f32)).reshape(KD, C)
    b0 = conv_b.astype(f32) @ embed_w.astype(f32) + embed_b.astype(f32)
    m1x = np.zeros((KP, C), dtype=f32)
    m1x[:KD] = m1
    m1x[KD] = b0
    shared = {
        "m1": m1x.astype(bf16),
        "m1w1": (m1x @ cheb0_w1.astype(f32)).astype(bf16),
        "w0_0": cheb0_w0.astype(bf16), "w0_1": cheb1_w0.astype(bf16),
        "w1b": cheb1_w1.astype(bf16),
        "mw1": mlp_w1.astype(bf16), "mw2": mlp_w2.astype(bf16),
        "cb_0": cheb0_b.reshape(C, 1).astype(f32),
        "cb_1": cheb1_b.reshape(C, 1).astype(f32),
        "mb1": mlp_b1.reshape(H, 1).astype(f32),
        "b2t": np.tile(mlp_b2.astype(f32).reshape(1, OPD), (P, 1)),
        "ident": np.eye(P, dtype=np.float32).astype(bf16),
    }
    return shared


def prepare(x, edge_index, conv_w, conv_b, embed_w, embed_b,
            cheb0_w0, cheb0_w1, cheb0_b, cheb1_w0, cheb1_w1, cheb1_b,
            mlp_w1, mlp_b1, mlp_w2, mlp_b2):
    """Host preprocessing: returns (compiled program, per-core in_maps)."""
    x = np.asarray(x, dtype=np.float32)
    idx_all, s_all, cols_rs, chunk_tile, n_round_a = _preprocess_graph(
        np.asarray(edge_index))

    shared = _prep_weights(
        np.asarray(conv_w, np.float32), np.asarray(conv_b, np.float32),
        np.asarray(embed_w, np.float32), np.asarray(embed_b, np.float32),
        np.asarray(cheb0_w0, np.float32), np.asarray(cheb0_w1, np.float32),
        np.asarray(cheb0_b, np.float32),
        np.asarray(cheb1_w0, np.float32), np.asarray(cheb1_w1, np.float32),
        np.asarray(cheb1_b, np.float32),
        np.asarray(mlp_w1, np.float32), np.asarray(mlp_b1, np.float32),
        np.asarray(mlp_w2, np.float32), np.asarray(mlp_b2, np.float32))
    shared.update({"idx": idx_all, "sall": s_all})

    # x: [B, T, E, D] -> [B, EP, 64] bf16: (t,i) flattened, ones col at 48
    # (carries the fused conv+embed bias), zero pad cols 49: and rows >= E.
    x2 = np.zeros((B, EP, KP), dtype=bf16)
    x2[:, :E, :KD] = x.transpose(0, 2, 1, 3).reshape(B, E, KD).astype(bf16)
    x2[:, :E, KD] = bf16(1.0)

    nc = _build_program(chunk_tile, n_round_a)

    in_maps = []
    for ci in range(NCORES):
        m = dict(shared)
        xs = x2[ci * BPC:(ci + 1) * BPC]
        m["xt"] = np.ascontiguousarray(xs.transpose(0, 2, 1))
        # layer-1 edge messages, host-gathered into chunk order:
        # em[p, c, :] = x2[:, cols[c*128+p], :] flattened over (b, k)
        xcat = np.ascontiguousarray(xs.transpose(1, 0, 2))  # [EP, BPC, KP]
        em = xcat[cols_rs]                    # [nch, P, BPC, KP]
        m["em"] = np.ascontiguousarray(
            em.transpose(1, 0, 2, 3).reshape(P, len(chunk_tile), BPC * KP))
        in_maps.append(m)
    return nc, in_maps


def kernel(**inputs):
    nc, in_maps = prepare(**inputs)
    res = run_bass_kernel_spmd(nc, in_maps, list(range(NCORES)))
    out = np.concatenate([res.results[ci]["out"] for ci in range(NCORES)],
                         axis=0)
    return np.ascontiguousarray(out, dtype=np.float32)


# revision 57
# speedup vs baseline: 1.3826x; 1.3057x over previous
"""GCN+MLP (ChebConv K=2, sym norm) Trainium2 Bass/Tile kernel.

nn_GCNMLP_81320910782821: out = MLP(relu(cheb1(relu(cheb0(embed(conv(x)))))))
with cheb(h) = h@W0 + (S@h)@W1 + b, S = -D^-1/2 A D^-1/2 (160k random edges,
E=10000 nodes, C=128 channels).

Sharding: data-parallel over batch B=32 -> 8 NeuronCores x 4 batch elems.
Per-core device kernel (bf16 compute, fp32 PSUM accumulation):
  * h kept SBUF-resident transposed: hT [C=128 part, 4b, E] bf16.
  * The segment-sum is one-hot matmul scatter over 128-edge chunks grouped
    by 128-row dest tiles; the one-hot matrices S_chunk[e,d] = w_e*(row_e==d)
    are host-precomputed and streamed from DRAM.
  * Layer 1 aggregates in x-space ((S@h0)@W1 == (S@X2)@(M1@W1)), so its
    messages are a pure input-layout transform that the host pre-gathers
    into edge order (em table) -> no device gather at all in layer 1.
  * Layer 2 messages are device-gathered from a row-major copy of h1 (h4,
    [E, 4b*C] bf16, 1024B rows) via dma_gather across 4 SWDGE queues.
  * Each dest tile's edges are split into two rounds by source node
    (< / >= MID): round-A of layer 2 only reads h4[:MID], so its gathers and
    scatters overlap the back half of layer 1; round partial sums round-trip
    through a DRAM buffer and are re-added on DVE.
  * Output uses the e = 79*p + j partition mapping so the final
    [N_PRED, E, PD] DMA has 1264B contiguous runs.

kernel(**inputs) takes FULL unsharded fp32/int64 inputs and returns the FULL
[B, N_PRED, E, PD] fp32 output. The Bass program is input-shape static but
depends on the per-tile chunk counts of the actual graph; it is built and
compiled on first call (cached per chunk signature).
"""

import functools

import numpy as np
import ml_dtypes

import concourse.bacc as bacc
import concourse.bass as bass
import concourse.mybir as mybir
import concourse.tile as tile
from concourse.bass_utils import run_bass_kernel_spmd

B, T, E, D = 32, 12, 10000, 4
C, H = 128, 64
N_PRED, PD = 12, 4
NCORES = 8
BPC = B // NCORES          # batch elems per core
P = 128
NJ = 79                    # e = NJ*p + j partition mapping
EP = P * NJ                # 10112 (E padded)
NT = EP // P               # 79 dest tiles of 128 rows
KD = T * D                 # 48 contraction dim of fused conv+embed
KDX = KD + 1               # + ones column carrying the fused bias
KP = 64                    # KDX padded (layer-1 message row 4*64*2B = 512B)
GG = 8                     # chunks per message group
OPD = N_PRED * PD          # 48
E_MAIN = (E // NJ) * NJ    # 9954 = 126*79 (rows covered by partitions 0..125)
MID = 40 * P               # 5120: source split point (h4-tile aligned)

BF = mybir.dt.bfloat16
F32 = mybir.dt.float32
I16 = mybir.dt.int16
AF = mybir.ActivationFunctionType
ALU = mybir.AluOpType
bf16 = ml_dtypes.bfloat16


# ---------------------------------------------------------------- host side

def _preprocess_graph(edge_index):
    """Sort edges by (dest tile, source round, dest row); pack into 128-edge
    chunks. Round A = sources < MID, round B = rest; within each round the
    chunk stream is dest-tile contiguous.

    Returns (idx_all [128, nch*8] i16, s_all [128, nch, 128] bf16,
             cols_rs [nch, 128] i64, chunk_tile tuple, n_round_a int).
    """
    row = np.asarray(edge_index[0], dtype=np.int64)
    col = np.asarray(edge_index[1], dtype=np.int64)
    deg = np.bincount(row, minlength=E).astype(np.float32)
    dis = np.where(deg > 0, 1.0 / np.sqrt(np.maximum(deg, 1.0)), 0.0).astype(np.float32)
    w = (-dis[row] * dis[col]).astype(np.float32)
    order = np.argsort(row, kind="stable")
    r_s, c_s, w_s = row[order], col[order], w[order]
    bounds = np.searchsorted(r_s, np.arange(NT + 1) * P)

    cols_p, rloc_p, ws_p, tl = [], [], [], []
    for t in range(NT):
        s, e_ = int(bounds[t]), int(bounds[t + 1])
        tc, tr, tw = c_s[s:e_], r_s[s:e_], w_s[s:e_]
        n = len(tc)
        nch_t = max(1, -(-n // P))
        pad = nch_t * P - n
        cols_p.append(np.pad(tc, (0, pad)))
        rloc_p.append(np.pad(tr - t * P, (0, pad)))
        ws_p.append(np.pad(tw, (0, pad)))
        tl += [t] * nch_t

    cols = np.concatenate(cols_p).astype(np.int16)
    rloc = np.concatenate(rloc_p).astype(np.int64)
    ws = np.concatenate(ws_p).astype(np.float32)
    chunk_tile = tuple(tl)
    n_round_a = 0
    nch = len(chunk_tile)
    # dma_gather index layout: idx i at [partition i%16, col i//16], x8 replicas
    idx_all = np.tile(cols.reshape(nch * 8, 16).T, (8, 1)).astype(np.int16)
    # one-hot scatter matrices, laid out [p(edge-in-chunk), chunk, dest]
    s_all = np.zeros((P, nch, P), dtype=bf16)
    cc, pp = np.meshgrid(np.arange(nch), np.arange(P), indexing="ij")
    s_all[pp.ravel(), cc.ravel(), rloc.reshape(nch, P).ravel()] = \
        ws.reshape(nch, P).ravel()
    return (idx_all, s_all, cols.reshape(nch, P).astype(np.int64),
            chunk_tile, n_round_a)


# ------------------------------------------------------------- device build

@functools.lru_cache(maxsize=2)
def _build_program(chunk_tile, n_round_a):
    nch = len(chunk_tile)
    nc = bacc.Bacc("TRN2", target_bir_lowering=False, debug=False,
                   num_devices=NCORES, num_swdge_queues=4)

    xt_in = nc.dram_tensor("xt", [BPC, KP, EP], BF, kind="ExternalInput")
    em_in = nc.dram_tensor("em", [P, nch, BPC * KP], BF, kind="ExternalInput")
    idx_in = nc.dram_tensor("idx", [P, nch * 8], I16, kind="ExternalInput")
    s_in = nc.dram_tensor("sall", [P, nch, P], BF, kind="ExternalInput")
    ident_in = nc.dram_tensor("ident", [P, P], BF, kind="ExternalInput")
    m1_in = nc.dram_tensor("m1", [KP, C], BF, kind="ExternalInput")
    m1w1_in = nc.dram_tensor("m1w1", [KP, C], BF, kind="ExternalInput")
    w0_in = [nc.dram_tensor(f"w0_{l}", [C, C], BF, kind="ExternalInput") for l in range(2)]
    w1b_in = nc.dram_tensor("w1b", [C, C], BF, kind="ExternalInput")
    mw1_in = nc.dram_tensor("mw1", [C, H], BF, kind="ExternalInput")
    mw2_in = nc.dram_tensor("mw2", [H, OPD], BF, kind="ExternalInput")
    cb_in = [nc.dram_tensor(f"cb_{l}", [C, 1], F32, kind="ExternalInput") for l in range(2)]
    mb1_in = nc.dram_tensor("mb1", [H, 1], F32, kind="ExternalInput")
    b2_in = nc.dram_tensor("b2t", [P, OPD], F32, kind="ExternalInput")
    out_ext = nc.dram_tensor("out", [BPC, N_PRED, E, PD], F32, kind="ExternalOutput")
    h4 = nc.dram_tensor("h4", [EP, BPC * C], BF)

    def make_groups(c0, c1):
        return [(i, min(i + GG, c1)) for i in range(c0, c1, GG)]

    with tile.TileContext(nc) as tc:
        with tc.tile_pool(name="const", bufs=1) as cp, \
             tc.tile_pool(name="work", bufs=2) as wp, \
             tc.tile_pool(name="psA", bufs=2, space="PSUM") as psA, \
             tc.tile_pool(name="psB", bufs=2, space="PSUM") as psB:

            def const_sb(handle, shape, dtype):
                t_ = cp.tile(shape, dtype, name=handle.name + "_sb")
                nc.sync.dma_start(t_, handle.ap())
                return t_

            idx_sb = const_sb(idx_in, [P, nch * 8], I16)
            ident_sb = const_sb(ident_in, [P, P], BF)
            m1_sb = const_sb(m1_in, [KP, C], BF)
            m1w1_sb = const_sb(m1w1_in, [KP, C], BF)
            w0_sb = [const_sb(w0_in[l], [C, C], BF) for l in range(2)]
            w1b_sb = const_sb(w1b_in, [C, C], BF)
            mw1_sb = const_sb(mw1_in, [C, H], BF)
            mw2_sb = const_sb(mw2_in, [H, OPD], BF)
            cb_sb = [const_sb(cb_in[l], [C, 1], F32) for l in range(2)]
            mb1_sb = const_sb(mb1_in, [H, 1], F32)
            b2_sb = const_sb(b2_in, [P, OPD], F32)

            hT = cp.tile([P, BPC, EP], BF, name="hT")
            xt_ap = xt_in.ap()
            s_ap = s_in.ap()
            em_ap = em_in.ap()
            idx_ap = idx_in.ap()
            h4_ap = h4.ap()

            # ---- phase A: h0^T = M1x^T @ x2^T (fused conv+embed+bias) ----
            for b in range(BPC):
                xT = wp.tile([KP, EP], BF, tag="zT", bufs=1)
                nc.sync.dma_start(xT, xt_ap[b])
                for ws in range(0, EP, 512):
                    we = min(ws + 512, EP)
                    ph = psA.tile([C, 512], F32, tag="ps_sc", bufs=3)
                    nc.tensor.matmul(ph[:, :we - ws], m1_sb, xT[:, ws:we],
                                     start=True, stop=True)
                    nc.scalar.copy(hT[:, b, ws:we], ph[:, :we - ws])

            # ---- scatter machinery (one tile-major round per layer) ----
            def scatter_round(l, finish):
                """One layer's scatter stream.

                l: 0 = x-space messages from the host em table (plain DMA),
                   1 = gather h1 rows from h4.
                """
                fw = KP if l == 0 else C
                elem = BPC * fw
                ps = None
                for gi, (c0, c1) in enumerate(make_groups(0, nch)):
                    G = c1 - c0
                    msgs = wp.tile([P, GG, elem], BF, tag="mB", bufs=4)
                    if l == 0:
                        nc.sync.dma_start(msgs[:, :G, :], em_ap[:, c0:c1, :])
                    else:
                        nc.gpsimd.dma_gather(
                            out_ap=msgs[:, :G, :],
                            in_ap=h4_ap,
                            idxs_ap=idx_sb[:, c0 * 8:c1 * 8],
                            num_idxs=G * P,
                            num_idxs_reg=G * P,
                            elem_size=elem,
                            queue_num=gi % 4,
                        )
                    S_sb = wp.tile([P, GG, P], BF, tag="SB", bufs=4)
                    nc.sync.dma_start(S_sb[:, :G, :], s_ap[:, c0:c1, :])
                    for ci in range(c0, c1):
                        t = chunk_tile[ci]
                        first = ci == 0 or chunk_tile[ci - 1] != t
                        last = ci == nch - 1 or chunk_tile[ci + 1] != t
                        if first:
                            ps = psA.tile([P, BPC * C], F32, tag="ps_sc",
                                          bufs=3)
                        k = ci - c0
                        nc.tensor.matmul(
                            ps[:, :elem], S_sb[:, k, :], msgs[:, k, :],
                            start=first, stop=last)
                        if last:
                            finish(t, ps, fw)

            # finisher: evac, transpose, dense, relu, (h4 table for l=0)
            def finish_full(l, t, ps, fw):
                tx1r = wp.tile([P, BPC * C], BF, tag="tx1r", bufs=2)
                nc.vector.tensor_copy(tx1r[:, :BPC * fw], ps[:, :BPC * fw])
                tx1T = wp.tile([C, BPC, P], BF, tag="tx1T", bufs=2)
                for b in range(BPC):
                    ptt = psB.tile([C, P], BF, tag="ps_tr", bufs=3)
                    nc.tensor.transpose(ptt[:fw, :], tx1r[:, b * fw:(b + 1) * fw],
                                        ident_sb)
                    nc.vector.tensor_copy(tx1T[:fw, b, :], ptt[:fw, :])
                wagg = m1w1_sb if l == 0 else w1b_sb
                for b in range(BPC):
                    pd = psB.tile([C, P], F32, tag="ps_d", bufs=2)
                    nc.tensor.matmul(pd, w0_sb[l], hT[:, b, t * P:(t + 1) * P],
                                     start=True, stop=False)
                    nc.tensor.matmul(pd, wagg[:fw, :], tx1T[:fw, b, :],
                                     start=False, stop=True)
                    nc.scalar.activation(hT[:, b, t * P:(t + 1) * P], pd,
                                         AF.Relu, bias=cb_sb[l], scale=1.0)
                if l == 0:
                    stag = wp.tile([P, BPC * C], BF, tag="stagA", bufs=3)
                    for b in range(BPC):
                        pt = psB.tile([P, C], BF, tag="ps_tr", bufs=3)
                        nc.tensor.transpose(pt, hT[:, b, t * P:(t + 1) * P],
                                            ident_sb)
                        nc.vector.tensor_copy(stag[:, b * C:(b + 1) * C], pt)
                    nc.sync.dma_start(h4_ap[t * P:(t + 1) * P, :], stag)

            for l in range(2):
                scatter_round(l, lambda t, ps, fw, l=l: finish_full(l, t, ps, fw))

            # ---- MLP + output ----
            out_ap = out_ext.ap()
            for b in range(BPC):
                zT = wp.tile([H, EP], BF, tag="zT", bufs=1)
                for ws in range(0, EP, 512):
                    we = min(ws + 512, EP)
                    pm = psA.tile([H, 512], F32, tag="ps_sc", bufs=3)
                    nc.tensor.matmul(pm[:, :we - ws], mw1_sb, hT[:, b, ws:we],
                                     start=True, stop=True)
                    nc.scalar.activation(zT[:, ws:we], pm[:, :we - ws],
                                         AF.Relu, bias=mb1_sb, scale=1.0)
                stagP = wp.tile([P, N_PRED, NJ, PD], BF, tag="stagP", bufs=2)
                zTb = zT.rearrange("h (q j) -> h j q", j=NJ)
                for j in range(NJ):
                    pp = psB.tile([P, OPD], F32, tag="ps_d", bufs=2)
                    nc.tensor.matmul(pp, zTb[:, j, :], mw2_sb,
                                     start=True, stop=True)
                    nc.vector.tensor_tensor(
                        out=stagP[:, :, j, :],
                        in0=pp.rearrange("p (n c) -> p n c", n=N_PRED),
                        in1=b2_sb.rearrange("p (n c) -> p n c", n=N_PRED),
                        op=ALU.add)
                out_b = out_ap[b]
                main = out_b[:, :E_MAIN, :].rearrange("n (p j) c -> p n j c", j=NJ)
                nc.gpsimd.dma_start(out=main, in_=stagP[:E_MAIN // NJ])
                tail = out_b[:, E_MAIN:E, :].rearrange("n (p j) c -> p n j c", p=1)
                nc.gpsimd.dma_start(
                    out=tail, in_=stagP[E_MAIN // NJ:E_MAIN // NJ + 1, :, :E - E_MAIN, :])

    nc.compile()
    return nc


# ----------------------------------------------------------------- kernel()

def _prep_weights(conv_w, conv_b, embed_w, embed_b,
                  cheb0_w0, cheb0_w1, cheb0_b, cheb1_w0, cheb1_w1, cheb1_b,
                  mlp_w1, mlp_b1, mlp_w2, mlp_b2):
    f32 = np.float32
    m1 = np.einsum("oit,oc->tic", conv_w.astype(f32),
                   embed_w.astype(f32)).reshape(KD, C)
    b0 = conv_b.astype(f32) @ embed_w.astype(f32) + embed_b.astype(f32)
    m1x = np.zeros((KP, C), dtype=f32)
    m1x[:KD] = m1
    m1x[KD] = b0
    shared = {
        "m1": m1x.astype(bf16),
        "m1w1": (m1x @ cheb0_w1.astype(f32)).astype(bf16),
        "w0_0": cheb0_w0.astype(bf16), "w0_1": cheb1_w0.astype(bf16),
        "w1b": cheb1_w1.astype(bf16),
        "mw1": mlp_w1.astype(bf16), "mw2": mlp_w2.astype(bf16),
        "cb_0": cheb0_b.reshape(C, 1).astype(f32),
        "cb_1": cheb1_b.reshape(C, 1).astype(f32),
        "mb1": mlp_b1.reshape(H, 1).astype(f32),
        "b2t": np.tile(mlp_b2.astype(f32).reshape(1, OPD), (P, 1)),
        "ident": np.eye(P, dtype=np.float32).astype(bf16),
    }
    return shared


def prepare(x, edge_index, conv_w, conv_b, embed_w, embed_b,
            cheb0_w0, cheb0_w1, cheb0_b, cheb1_w0, cheb1_w1, cheb1_b,
            mlp_w1, mlp_b1, mlp_w2, mlp_b2):
    """Host preprocessing: returns (compiled program, per-core in_maps)."""
    x = np.asarray(x, dtype=np.float32)
    idx_all, s_all, cols_rs, chunk_tile, n_round_a = _preprocess_graph(
        np.asarray(edge_index))

    shared = _prep_weights(
        np.asarray(conv_w, np.float32), np.asarray(conv_b, np.float32),
        np.asarray(embed_w, np.float32), np.asarray(embed_b, np.float32),
        np.asarray(cheb0_w0, np.float32), np.asarray(cheb0_w1, np.float32),
        np.asarray(cheb0_b, np.float32),
        np.asarray(cheb1_w0, np.float32), np.asarray(cheb1_w1, np.float32),
        np.asarray(cheb1_b, np.float32),
        np.asarray(mlp_w1, np.float32), np.asarray(mlp_b1, np.float32),
        np.asarray(mlp_w2, np.float32), np.asarray(mlp_b2, np.float32))
    shared.update({"idx": idx_all, "sall": s_all})

    # x: [B, T, E, D] -> [B, EP, 64] bf16: (t,i) flattened, ones col at 48
    # (carries the fused conv+embed bias), zero pad cols 49: and rows >= E.
    x2 = np.zeros((B, EP, KP), dtype=bf16)
    x2[:, :E, :KD] = x.transpose(0, 2, 1, 3).reshape(B, E, KD).astype(bf16)
    x2[:, :E, KD] = bf16(1.0)

    nc = _build_program(chunk_tile, n_round_a)

    in_maps = []
    for ci in range(NCORES):
        m = dict(shared)
        xs = x2[ci * BPC:(ci + 1) * BPC]
        m["xt"] = np.ascontiguousarray(xs.transpose(0, 2, 1))
        # layer-1 edge messages, host-gathered into chunk order:
        # em[p, c, :] = x2[:, cols[c*128+p], :] flattened over (b, k)
        xcat = np.ascontiguousarray(xs.transpose(1, 0, 2))  # [EP, BPC, KP]
        em = xcat[cols_rs]                    # [nch, P, BPC, KP]
        m["em"] = np.ascontiguousarray(
            em.transpose(1, 0, 2, 3).reshape(P, len(chunk_tile), BPC * KP))
        in_maps.append(m)
    return nc, in_maps


def kernel(**inputs):
    nc, in_maps = prepare(**inputs)
    res = run_bass_kernel_spmd(nc, in_maps, list(range(NCORES)))
    out = np.concatenate([res.results[ci]["out"] for ci in range(NCORES)],
                         axis=0)
    return np.ascontiguousarray(out, dtype=np.float32)
